# revision 1
# baseline (speedup 1.0000x reference)
"""Two-phase banded-group Trainium2 kernel for nn_ClementsBellNxN (N=512).

Phase A (vector engines, band coordinates): each core builds 2 group
matrices B_j = prod of 16 consecutive fused steps S_i, stored as banded
matrices (halfwidth <= 32) in the pair-partition layout: row r = 4p+2b+v
(p partition, b block, v: 0=T even-row, 1=U odd-row), free axis = diagonal
offset d (D=70 slots, diagonal at OFF=35).  A step couples rows at distance
<= 2, so band ops are the baseline butterfly with +-1 free-dim shifts on the
cross terms.  256 steps / 8 cores = 32 step-applications per core at band
width instead of 256 at column width.

Host: decodes the 16 bands, scatters them into dense transposed 128x128
block tiles (pure layout transform, no arithmetic).

Phase B (PE): each core applies the 16 group matrices sequentially to its
64 columns of diag(e^{i ph0}) as block-tridiagonal fp16 matmuls with PSUM
accumulation (~40 matmuls/group), then the final row-phase rotation.
"""
import numpy as np

N = 512
S = 256
NCORES = 8
COLS = N // NCORES          # 64
G = 16                      # steps per group
NG = S // G                 # 16 groups
GPC = NG // NCORES          # 2 groups per core
SPC = G * GPC               # 32 steps per core
D = 70                      # band slots
OFF = 35                    # diagonal position
IL = 0.05
IMB = 0.005
_sq = np.sqrt(1.0 - IL)
A = np.float64(np.float32(_sq * np.sqrt(0.5 + IMB)))
B = np.float64(np.float32(_sq * np.sqrt(0.5 - IMB)))
# each step attenuates amplitudes by (1-IL)^2; rescale each group band by
# (1-IL)^(-2G) at build time (identity seed) and undo the total factor
# (1-IL)^(2*S) in the final rotation coefficients, keeping fp16 state O(1)
SCALE_INIT = float((1.0 - IL) ** (-2 * G))
UNSCALE = float((1.0 - IL) ** (2 * S))

# channel offsets in a state slab [128, 8*D] (re/im adjacent per (v,b))
CH_TRE0, CH_TIM0 = 0 * D, 1 * D
CH_TRE1, CH_TIM1 = 2 * D, 3 * D
CH_URE0, CH_UIM0 = 4 * D, 5 * D
CH_URE1, CH_UIM1 = 6 * D, 7 * D
SLAB = 8 * D

# ---------------------------------------------------------------- host math


def _fused2x2(ph_first, ph_second):
    p = np.exp(1j * np.float64(ph_first))
    q = np.exp(1j * np.float64(ph_second))
    alpha = A * A * p - B * B * q
    beta = 1j * A * B * (p + q)
    delta = A * A * q - B * B * p
    return alpha, beta, delta


def _pack6(dst, aa, bb, dd):
    amb, dmb = aa - bb, dd - bb
    dst[:, 0] = bb.real
    dst[:, 1] = bb.imag
    dst[:, 2] = -bb.real
    dst[:, 3] = amb.real
    dst[:, 4] = amb.imag
    dst[:, 5] = -amb.real
    dst[:, 6] = dmb.real
    dst[:, 7] = dmb.imag
    dst[:, 8] = -dmb.real


def _precompute(phases, nsteps):
    ph = np.float64(phases)
    k = np.arange(256)
    j = np.arange(128)
    ceven = np.zeros((128, nsteps, 2, 9), np.float64)
    codd = np.zeros((128, nsteps, 2, 9), np.float64)
    for i in range(nsteps):
        pa = ph[1 + 2 * i]
        pb = ph[2 + 2 * i]
        al, be, de = _fused2x2(pa[2 * k], pa[2 * k + 1])
        for b in range(2):
            sel = 2 * j + b
            _pack6(ceven[:, i, b], al[sel], be[sel], de[sel])
        ko = np.arange(255)
        alo, beo, deo = _fused2x2(pb[2 * ko + 1], pb[2 * ko + 2])
        alo = np.concatenate([alo, [0.0 + 0j]])
        beo = np.concatenate([beo, [0.0 + 0j]])
        deo = np.concatenate([deo, [0.0 + 0j]])
        _pack6(codd[:, i, 0], alo[2 * j], beo[2 * j], deo[2 * j])
        sel1 = np.minimum(2 * j + 1, 255)
        a1, b1_, d1 = alo[sel1].copy(), beo[sel1].copy(), deo[sel1].copy()
        a1[127] = np.exp(1j * pb[511])   # row 511 rotation (t-role lane)
        b1_[127] = 0.0
        d1[127] = np.exp(1j * pb[0])     # row 0 rotation (u-role via Pbwd)
        _pack6(codd[:, i, 1], a1, b1_, d1)
    pfwd = np.zeros((128, 128), np.float32)
    pfwd[np.arange(1, 128), np.arange(0, 127)] = 1.0
    pfwd[0, 127] = 1.0
    pbwd = np.zeros((128, 128), np.float32)
    pbwd[np.arange(0, 127), np.arange(1, 128)] = 1.0
    pbwd[127, 0] = 1.0
    return (ceven.reshape(128, nsteps * 18).astype(np.float32),
            codd.reshape(128, nsteps * 18).astype(np.float32),
            pfwd, pbwd)


# ---------------------------------------------------------------- bass build

_CACHE = {}
_CMUL = []


def _ensure_cmul_op():
    """Custom DVE op: out = C0*Src0 - C1*Src1 (per-partition scalars)."""
    if _CMUL:
        return _CMUL[0]
    import concourse.dve_ops as Dv
    from concourse.dve_spec import Src0, Src1, C0, C1, lower, _has_src1
    from concourse.dve_uop import DveOpSpec
    from concourse.dve_table_gen import dve_ver_for

    name = "CMUL_SUB_ANT"
    for o in Dv.OPS:
        if o.name == name:
            _CMUL.append(o)
            return o
    spec = Dv.Spec(body=(Src0 * C0) - (Src1 * C1), accum=None, accum_init=None,
                   reference=lambda in0, in1, c0, c1, c2: in0 * c0 - in1 * c1)
    ver = dve_ver_for("TRN2")
    opcode = 1 + len(Dv.OPS)
    tmp = DveOpSpec(name=name, opcode=opcode, uops=lower(spec, ver=ver),
                    rd1_en=_has_src1(spec))
    op = Dv.DveOp(name=name, spec=spec, subdim=False,
                  uops_sha={ver: tmp.sha(ver)})
    Dv.OPS.append(op)
    Dv._SUB_OPCODE_FOR_NAME[name] = opcode
    Dv.CUSTOM_DVE_SPECS[name] = spec
    _CMUL.append(op)
    return op


def _build_A():
    """Phase A: band-build GPC groups of G steps each."""
    import concourse.mybir as mybir
    from concourse import bacc, tile

    f32 = mybir.dt.float32
    f16 = mybir.dt.float16
    add = mybir.AluOpType.add
    sub = mybir.AluOpType.subtract
    mul = mybir.AluOpType.mult

    nc = bacc.Bacc("TRN2", target_bir_lowering=False, debug=False,
                   enable_asserts=False)
    cev_d = nc.dram_tensor("cev", [128, SPC * 18], f32, kind="ExternalInput")
    cod_d = nc.dram_tensor("cod", [128, SPC * 18], f32, kind="ExternalInput")
    pf_d = nc.dram_tensor("pf", [128, 128], f32, kind="ExternalInput")
    pb_d = nc.dram_tensor("pb", [128, 128], f32, kind="ExternalInput")
    out_d = nc.dram_tensor("bands", [128, GPC * SLAB], f32,
                           kind="ExternalOutput")

    cmul_op = _ensure_cmul_op()

    with tile.TileContext(nc) as tc:
        with (
            tc.tile_pool(name="coef", bufs=1) as cpool,
            tc.tile_pool(name="tmp", bufs=6) as tpool,
            tc.tile_pool(name="psum", bufs=2, space="PSUM") as ppool,
        ):
            cev = cpool.tile([128, SPC * 18], f32, tag="cev")
            cod = cpool.tile([128, SPC * 18], f32, tag="cod")
            pf = cpool.tile([128, 128], f32, tag="pf")
            pb = cpool.tile([128, 128], f32, tag="pb")
            nc.sync.dma_start(out=cev[:], in_=cev_d.ap())
            nc.sync.dma_start(out=cod[:], in_=cod_d.ap())
            nc.sync.dma_start(out=pf[:], in_=pf_d.ap())
            nc.sync.dma_start(out=pb[:], in_=pb_d.ap())
            # per group: three persistent state slabs (cur -> nxt -> nx2)
            # plus a persistent tt tile (its full width feeds the Pbwd
            # matmul, so never-written borders must stay zero)
            slabs = [[cpool.tile([128, SLAB], f32, tag=f"slab{g}_{i}",
                                 name=f"slab{g}_{i}") for i in range(3)]
                     for g in range(GPC)]
            ttiles = [cpool.tile([128, 2 * D], f32, tag=f"ttile{g}",
                                 name=f"ttile{g}") for g in range(GPC)]

            def cmul(out, i0, i1, sc0, sc1):
                nc.vector._custom_dve(cmul_op, out=out, in0=i0, in1=i1,
                                      s0=sc0, s1=sc1)

            def bh(tin, uin, coef, cb, tout, uout, W0, W1, peng, tg,
                   seng=None, ueng=None, t_scheme_a=False):
                """Band half-block: t' = (a-b) t + m, u' = (d-b) u + m[+1],
                m = b*s, s = t + u[-1].  tin/uin/tout/uout = (tile, off_re,
                off_im) with off_im == off_re + D; coef cols cb..cb+8.
                Adds are re/im-fused 2-level-AP tensor_tensor ops; peng
                picks the engine (nc.gpsimd or nc.vector) for the fused
                z+m adds (s-add always on peng's complement)."""
                from concourse.ap import AP as _AP

                def ap2(tile_, off, gap, w_):
                    base = tile_[:]
                    return _AP(base.tensor, off,
                               [[base.ap[0][0], 128], [gap, 2], [1, w_]])

                w1s = W1 + 1
                w = W1 - W0
                br = coef[:, cb + 0:cb + 1]
                bi = coef[:, cb + 1:cb + 2]
                nbr = coef[:, cb + 2:cb + 3]
                ar = coef[:, cb + 3:cb + 4]
                ai = coef[:, cb + 4:cb + 5]
                nar = coef[:, cb + 5:cb + 6]
                dr = coef[:, cb + 6:cb + 7]
                di = coef[:, cb + 7:cb + 8]
                ndr = coef[:, cb + 8:cb + 9]
                g = nc.gpsimd
                v = nc.vector
                aeng = g if peng is v else v
                seng = seng or aeng
                ueng = ueng or aeng
                s_ = tpool.tile([128, 2 * D + 2], f32, tag="s" + tg,
                                name="s_t")
                m_ = tpool.tile([128, 2 * D + 2], f32, tag="m" + tg,
                                name="m_t")
                # s = t + u[-1], both channels in one fused TT
                seng.tensor_tensor(
                    out=ap2(s_, 0, D + 1, w + 1),
                    in0=ap2(tin[0], tin[1] + W0, D, w + 1),
                    in1=ap2(uin[0], uin[1] + W0 - 1, D, w + 1), op=add)
                sre = s_[:, 0:w + 1]
                sim = s_[:, D + 1:D + 2 + w]
                cmul(m_[:, 0:w + 1], sre, sim, br, bi)
                cmul(m_[:, D + 1:D + 2 + w], sre, sim, bi, nbr)
                tre = tin[0][:, tin[1] + W0:tin[1] + W1]
                tim = tin[0][:, tin[2] + W0:tin[2] + W1]
                ure = uin[0][:, uin[1] + W0:uin[1] + W1]
                uim = uin[0][:, uin[2] + W0:uin[2] + W1]
                z2 = tpool.tile([128, 2 * D], f32, tag="z2" + tg,
                                name="z2_t")
                if t_scheme_a:
                    # t' = amb*t + m via two chained Pool STTs per channel
                    otre = tout[0][:, tout[1] + W0:tout[1] + W1]
                    otim = tout[0][:, tout[2] + W0:tout[2] + W1]
                    stt = nc.gpsimd.scalar_tensor_tensor
                    stt(out=otre, in0=tim, scalar=ai, in1=m_[:, 0:w],
                        op0=mul, op1=sub)
                    stt(out=otre, in0=tre, scalar=ar, in1=otre,
                        op0=mul, op1=sub)
                    stt(out=otim, in0=tre, scalar=ai,
                        in1=m_[:, D + 1:D + 1 + w], op0=mul, op1=add)
                    stt(out=otim, in0=tim, scalar=ar, in1=otim,
                        op0=mul, op1=add)
                else:
                    z = tpool.tile([128, 2 * D], f32, tag="z" + tg,
                                   name="z_t")
                    cmul(z[:, 0:w], tre, tim, ar, ai)
                    cmul(z[:, D:D + w], tre, tim, ai, nar)
                    peng.tensor_tensor(
                        out=ap2(tout[0], tout[1] + W0, D, w),
                        in0=ap2(z, 0, D, w), in1=ap2(m_, 0, D + 1, w),
                        op=add)
                cmul(z2[:, 0:w], ure, uim, dr, di)
                cmul(z2[:, D:D + w], ure, uim, di, ndr)
                ueng.tensor_tensor(
                    out=ap2(uout[0], uout[1] + W0, D, w),
                    in0=ap2(z2, 0, D, w), in1=ap2(m_, 1, D + 1, w), op=add)

            # identity band init for both groups
            rot = []
            for grp in range(GPC):
                # spread the zero-fills across engines so they don't
                # serialize on Pool ahead of the first step's adds
                eng = nc.gpsimd if grp == 0 else nc.vector
                for sb in slabs[grp]:
                    eng.memset(sb[:], 0.0)
                nc.scalar.memzero(ttiles[grp][:])
                cur = slabs[grp][0]
                for ch in (CH_TRE0, CH_TRE1, CH_URE0, CH_URE1):
                    nc.vector.memset(cur[:, ch + OFF:ch + OFF + 1],
                                     SCALE_INIT)
                rot.append([cur, slabs[grp][1], slabs[grp][2]])

            def step_phases(grp, m_):
                """Six thunks for one build step of one group; interleaved
                across groups at thunk granularity so each group's
                dependency stalls are filled by the other's ops."""
                from concourse.ap import AP as _AP
                st = grp * G + m_
                tg = str(grp)
                cur, nxt, nx2 = rot[grp]
                ttile = ttiles[grp]
                W0e, W1e = OFF - 2 * m_ - 1, OFF + 2 * m_ + 2
                W0o, W1o = OFF - 2 * m_ - 2, OFF + 2 * m_ + 3
                v, g = nc.vector, nc.gpsimd
                box = {}

                def ev_pair():
                    # both even b-blocks with 4-channel fused adds:
                    # one s-add, one t-add, one u-add across b0|b1 re|im
                    from concourse.ap import AP as _AP

                    def ap3(tile_, off, gap, n_, w_):
                        base = tile_[:]
                        return _AP(base.tensor, off,
                                   [[base.ap[0][0], 128], [gap, n_],
                                    [1, w_]])

                    w = W1e - W0e
                    s4 = tpool.tile([128, 4 * (D + 1)], f32,
                                    tag="s4" + tg, name="s4")
                    m4 = tpool.tile([128, 4 * (D + 1)], f32,
                                    tag="m4" + tg, name="m4")
                    z4 = tpool.tile([128, 4 * D], f32, tag="z4" + tg,
                                    name="z4")
                    z24 = tpool.tile([128, 4 * D], f32, tag="z24" + tg,
                                     name="z24")
                    g.tensor_tensor(
                        out=ap3(s4, 0, D + 1, 4, w + 1),
                        in0=ap3(cur, CH_TRE0 + W0e, D, 4, w + 1),
                        in1=ap3(cur, CH_URE0 + W0e - 1, D, 4, w + 1),
                        op=add)
                    for b in range(2):
                        cb = (st * 2 + b) * 9
                        br = cev[:, cb + 0:cb + 1]
                        bi = cev[:, cb + 1:cb + 2]
                        nbr = cev[:, cb + 2:cb + 3]
                        ar = cev[:, cb + 3:cb + 4]
                        ai = cev[:, cb + 4:cb + 5]
                        nar = cev[:, cb + 5:cb + 6]
                        dr = cev[:, cb + 6:cb + 7]
                        di = cev[:, cb + 7:cb + 8]
                        ndr = cev[:, cb + 8:cb + 9]
                        o2 = 2 * b * (D + 1)
                        sre = s4[:, o2:o2 + w + 1]
                        sim = s4[:, o2 + D + 1:o2 + D + 2 + w]
                        cmul(m4[:, o2:o2 + w + 1], sre, sim, br, bi)
                        cmul(m4[:, o2 + D + 1:o2 + D + 2 + w], sre, sim,
                             bi, nbr)
                        cht = CH_TRE0 if b == 0 else CH_TRE1
                        chu = CH_URE0 if b == 0 else CH_URE1
                        tre = cur[:, cht + W0e:cht + W1e]
                        tim = cur[:, cht + D + W0e:cht + D + W1e]
                        ure = cur[:, chu + W0e:chu + W1e]
                        uim = cur[:, chu + D + W0e:chu + D + W1e]
                        oz = 2 * b * D
                        cmul(z4[:, oz:oz + w], tre, tim, ar, ai)
                        cmul(z4[:, oz + D:oz + D + w], tre, tim, ai, nar)
                        cmul(z24[:, oz:oz + w], ure, uim, dr, di)
                        cmul(z24[:, oz + D:oz + D + w], ure, uim, di, ndr)
                    g.tensor_tensor(
                        out=ap3(nxt, CH_TRE0 + W0e, D, 4, w),
                        in0=ap3(z4, 0, D, 4, w),
                        in1=ap3(m4, 0, D + 1, 4, w), op=add)
                    v.tensor_tensor(
                        out=ap3(nxt, CH_URE0 + W0e, D, 4, w),
                        in0=ap3(z24, 0, D, 4, w),
                        in1=ap3(m4, 1, D + 1, 4, w), op=add)

                def odd0():
                    bh((nxt, CH_URE0, CH_UIM0), (nxt, CH_TRE1, CH_TIM1),
                       cod, (st * 2) * 9,
                       (nx2, CH_URE0, CH_UIM0), (nx2, CH_TRE1, CH_TIM1),
                       W0o, W1o, g, tg, seng=g, ueng=g)

                def pe1():
                    tshp = ppool.tile([128, 2 * D], f32, tag="tshp" + tg,
                                      name="tshp")
                    nc.tensor.matmul(out=tshp[:], lhsT=pf[:],
                                     rhs=nxt[:, 0:2 * D],
                                     start=True, stop=True)
                    tshs = tpool.tile([128, 2 * D], f32, tag="tshs" + tg,
                                      name="tshs")
                    a0, a1 = W0o - 1, W1o + 1
                    base = tshs[:]
                    nc.scalar.copy(
                        _AP(base.tensor, a0,
                            [[base.ap[0][0], 128], [D, 2], [1, a1 - a0]]),
                        _AP(tshp[:].tensor, tshp[:].offset + a0,
                            [[tshp[:].ap[0][0], 128], [D, 2], [1, a1 - a0]]))
                    box["tshs"] = tshs

                def odd1():
                    bh((nxt, CH_URE1, CH_UIM1), (box["tshs"], 0, D),
                       cod, (st * 2 + 1) * 9,
                       (nx2, CH_URE1, CH_UIM1), (ttile, 0, D),
                       W0o, W1o, v, tg, seng=g, ueng=g)  # t-add on DVE

                def pe2():
                    t0p = ppool.tile([128, 2 * D], f32, tag="t0p" + tg,
                                     name="t0p")
                    nc.tensor.matmul(out=t0p[:], lhsT=pb[:], rhs=ttile[:],
                                     start=True, stop=True)
                    a0 = max(0, W0o - 3)
                    a1 = min(D, W1o + 3)
                    base = nx2[:]
                    nc.scalar.copy(
                        _AP(base.tensor, a0,
                            [[base.ap[0][0], 128], [D, 2], [1, a1 - a0]]),
                        _AP(t0p[:].tensor, t0p[:].offset + a0,
                            [[t0p[:].ap[0][0], 128], [D, 2], [1, a1 - a0]]))

                return [ev_pair, pe1, odd0, odd1, pe2]

            for m_ in range(G):
                phases = [step_phases(grp, m_) for grp in range(GPC)]
                for i in range(5):
                    for grp in range(GPC):
                        phases[grp][i]()
                for grp in range(GPC):
                    cur, nxt, nx2 = rot[grp]
                    rot[grp] = [nx2, cur, nxt]
            for grp in range(GPC):
                nc.sync.dma_start(
                    out=out_d.ap()[:, grp * SLAB:(grp + 1) * SLAB],
                    in_=rot[grp][0][:])
    nc.compile()
    return nc


def _build_B():
    """Phase B: apply NG banded group matrices to this core's 64 columns.

    X per row-block is one fp16 tile [128, 3*COLS] = [imneg | re | im], so
    each (r,k) block contributes two wide matmuls into a fused [re|im]
    PSUM tile:  ps += BreT' . [re|im]  and  ps += BimT' . [imneg|re].
    Corner blocks (r,k=r+-1) have nonzero contraction rows only in
    [0:32) / [96:128), packed two-per-tile at matching base partitions.
    """
    import concourse.mybir as mybir
    from concourse import bacc, tile

    f32 = mybir.dt.float32
    f16 = mybir.dt.float16

    nc = bacc.Bacc("TRN2", target_bir_lowering=False, debug=False,
                   enable_asserts=False)
    # per group 14 col-blocks of 128: 0..7 diag (2r+reim), 8..13 corner
    # tiles (2t+reim): up-corner (t,t+1) at partitions [0:32), down-corner
    # (t+1,t) at partitions [96:128)
    bk_d = nc.dram_tensor("blkT", [128, NG * 14 * 128], f16,
                          kind="ExternalInput")
    x0_d = nc.dram_tensor("x0", [128, 12 * COLS], f16, kind="ExternalInput")
    cf_d = nc.dram_tensor("cfb", [128, 12], f32, kind="ExternalInput")
    out_d = nc.dram_tensor("xout", [128, 8 * COLS], f32,
                           kind="ExternalOutput")

    cmul_op = _ensure_cmul_op()
    C3 = 3 * COLS

    with tile.TileContext(nc) as tc:
        with (
            tc.tile_pool(name="coef", bufs=4) as kpool,
            tc.tile_pool(name="state", bufs=1) as spool,
            tc.tile_pool(name="psum", bufs=2, space="PSUM") as ppool,
        ):
            cf = spool.tile([128, 12], f32, tag="cf")
            nc.sync.dma_start(out=cf[:], in_=cf_d.ap())
            gens = []
            for gi in range(3):
                gens.append([spool.tile([128, C3], f16,
                                        tag=f"x{gi}_{blk}",
                                        name=f"x{gi}_{blk}")
                             for blk in range(4)])
            x0 = spool.tile([128, 12 * COLS], f16, tag="x0")
            nc.sync.dma_start(out=x0[:], in_=x0_d.ap())
            for blk in range(4):
                nc.vector.tensor_scalar_mul(
                    out=gens[0][blk][:],
                    in0=x0[:, blk * C3:(blk + 1) * C3], scalar1=1.0)

            obuf = spool.tile([128, 8 * COLS], f32, tag="obuf")

            cur = 0
            for j in range(NG):
                bkt = kpool.tile([128, 1792], f16, tag="bk", name="bk")
                nc.sync.dma_start(
                    out=bkt[:], in_=bk_d.ap()[:, j * 1792:(j + 1) * 1792])
                bk = bkt[:]
                X = gens[cur]
                Y = gens[(cur + 1) % 3]
                for r in range(4):
                    ps = ppool.tile([128, 128], f32, tag=f"ps{r}",
                                    name=f"ps{r}")
                    # order contribs by when their X input drains
                    # (Y[r-1] lands before Y[r] before Y[r+1]) so each
                    # chain's first matmul can issue one drain earlier
                    contribs = []
                    if r > 0:  # down corner (r, r-1): nonzero rows
                        # [96:128) but PE base partition must be 0/32/64,
                        # so use [64:128) (rows 64:96 are zero-packed)
                        c0 = (8 + 2 * (r - 1)) * 128
                        contribs.append((bk[64:128, c0:c0 + 128],
                                         bk[64:128, c0 + 128:c0 + 256],
                                         X[r - 1], (64, 128)))
                    contribs.append((bk[:, (2 * r) * 128:(2 * r + 1) * 128],
                                     bk[:, (2 * r + 1) * 128:
                                        (2 * r + 2) * 128],
                                     X[r], None))
                    if r < 3:  # up corner (r, r+1), contraction rows [0:32)
                        c0 = (8 + 2 * r) * 128
                        contribs.append((bk[0:32, c0:c0 + 128],
                                         bk[0:32, c0 + 128:c0 + 256],
                                         X[r + 1], (0, 32)))
                    nct = len(contribs)
                    for i_, (bre, bim, xk, pr) in enumerate(contribs):
                        if pr is None:
                            r1 = xk[:, COLS:C3]
                            r2 = xk[:, 0:2 * COLS]
                        else:
                            r1 = xk[pr[0]:pr[1], COLS:C3]
                            r2 = xk[pr[0]:pr[1], 0:2 * COLS]
                        nc.tensor.matmul(out=ps[:], lhsT=bre, rhs=r1,
                                         start=(i_ == 0), stop=False)
                        nc.tensor.matmul(out=ps[:], lhsT=bim, rhs=r2,
                                         start=False, stop=(i_ == nct - 1))
                    # PSUM -> SBUF: [re|im] fused on DVE, imneg on Act
                    nc.vector.tensor_scalar_mul(out=Y[r][:, COLS:C3],
                                                in0=ps[:], scalar1=1.0)
                    nc.scalar.mul(Y[r][:, 0:COLS], ps[:, COLS:128], -1.0)
                cur = (cur + 1) % 3

            # final rotation per block: o = e^{i phf} * x (+ loss unscale)
            X = gens[cur]
            for r in range(4):
                cosc = cf[:, 3 * r + 0:3 * r + 1]
                sinc = cf[:, 3 * r + 1:3 * r + 2]
                ncos = cf[:, 3 * r + 2:3 * r + 3]
                ore = obuf[:, (r * 2 + 0) * COLS:(r * 2 + 1) * COLS]
                oim = obuf[:, (r * 2 + 1) * COLS:(r * 2 + 2) * COLS]
                xre = X[r][:, COLS:2 * COLS]
                xim = X[r][:, 2 * COLS:C3]
                nc.vector._custom_dve(cmul_op, out=ore, in0=xre, in1=xim,
                                      s0=cosc, s1=sinc)
                nc.vector._custom_dve(cmul_op, out=oim, in0=xre, in1=xim,
                                      s0=sinc, s1=ncos)
            nc.sync.dma_start(out=out_d.ap(), in_=obuf[:])
    nc.compile()
    return nc


def _get_modules():
    if "A" not in _CACHE:
        _CACHE["A"] = _build_A()
        _CACHE["B"] = _build_B()
    return _CACHE["A"], _CACHE["B"]


# ---------------------------------------------------------------- host glue

_ROWS = {}
for _b in range(2):
    _p = np.arange(128)
    _ROWS[(0, _b)] = 4 * _p + 2 * _b        # T rows
    _ROWS[(1, _b)] = 4 * _p + 2 * _b + 1    # U rows

_CH_OFFS = [(CH_TRE0, 0, 0), (CH_TRE1, 0, 1), (CH_URE0, 1, 0),
            (CH_URE1, 1, 1)]


def _decode_band(slab):
    """slab [128, SLAB] float -> dense complex64 [512, 512]."""
    Bp = np.zeros((N, N + 2 * D), np.complex64)
    dd = np.arange(D)
    for off, v, b in _CH_OFFS:
        rows = _ROWS[(v, b)]
        re = slab[:, off:off + D].astype(np.float32)
        im = slab[:, off + D:off + 2 * D].astype(np.float32)
        cols = rows[:, None] + dd[None, :] - OFF + D
        Bp[rows[:, None], cols] = re + 1j * im
    return Bp[:, D:D + N]


def _pack_phaseB(Bs):
    """Bs: list of NG dense [512,512] complex64 -> blkT array (fp16)."""
    blkT = np.zeros((128, NG * 14 * 128), np.float16)
    for j in range(NG):
        Bj = Bs[j]
        g0 = j * 14 * 128
        for r in range(4):
            bT = Bj[r * 128:(r + 1) * 128, r * 128:(r + 1) * 128].T
            c0 = g0 + (2 * r) * 128
            blkT[:, c0:c0 + 128] = bT.real.astype(np.float16)
            blkT[:, c0 + 128:c0 + 256] = bT.imag.astype(np.float16)
        for t in range(3):
            c0 = g0 + (8 + 2 * t) * 128
            up = Bj[t * 128:(t + 1) * 128,
                    (t + 1) * 128:(t + 1) * 128 + 32].T      # [32, 128]
            dn = Bj[(t + 1) * 128:(t + 2) * 128,
                    t * 128 + 96:(t + 1) * 128].T            # [32, 128]
            blkT[0:32, c0:c0 + 128] = up.real.astype(np.float16)
            blkT[0:32, c0 + 128:c0 + 256] = up.imag.astype(np.float16)
            blkT[96:128, c0:c0 + 128] = dn.real.astype(np.float16)
            blkT[96:128, c0 + 128:c0 + 256] = dn.imag.astype(np.float16)
    return blkT


def _x0_for_core(phases, c):
    """Initial X = diag(e^{i ph0})[:, cols] in block layout + imneg."""
    ph0 = np.float64(phases[0])
    x0 = np.zeros((128, 12 * COLS), np.float16)
    for col in range(c * COLS, (c + 1) * COLS):
        row = col
        blk, p = row // 128, row % 128
        cc = col - c * COLS
        x0[p, (blk * 3 + 0) * COLS + cc] = np.float16(-np.sin(ph0[row]))
        x0[p, (blk * 3 + 1) * COLS + cc] = np.float16(np.cos(ph0[row]))
        x0[p, (blk * 3 + 2) * COLS + cc] = np.float16(np.sin(ph0[row]))
    return x0


def _cfb(phases):
    phf = np.float64(phases[N + 1])
    cf = np.zeros((128, 12), np.float32)
    p = np.arange(128)
    for blk in range(4):
        r = blk * 128 + p
        cf[:, 3 * blk + 0] = UNSCALE * np.cos(phf[r])
        cf[:, 3 * blk + 1] = UNSCALE * np.sin(phf[r])
        cf[:, 3 * blk + 2] = -UNSCALE * np.cos(phf[r])
    return cf


# ---------------------------------------------------------------- entry


def kernel(phases: np.ndarray) -> np.ndarray:
    from concourse.bass_utils import run_bass_kernel_spmd

    phases = np.asarray(phases)
    ncA, ncB = _get_modules()
    ce, co, pfwd, pbwd = _precompute(phases, S)

    in_maps = []
    for c in range(NCORES):
        s0 = c * SPC
        in_maps.append({
            "cev": ce[:, s0 * 18:(s0 + SPC) * 18].copy(),
            "cod": co[:, s0 * 18:(s0 + SPC) * 18].copy(),
            "pf": pfwd, "pb": pbwd,
        })
    resA = run_bass_kernel_spmd(ncA, in_maps, core_ids=list(range(NCORES)))

    Bs = []
    for c in range(NCORES):
        slab2 = resA.results[c]["bands"]
        for g in range(GPC):
            Bs.append(_decode_band(slab2[:, g * SLAB:(g + 1) * SLAB]))

    blkT = _pack_phaseB(Bs)
    cfb = _cfb(phases)
    in_maps = []
    for c in range(NCORES):
        in_maps.append({
            "blkT": blkT,
            "x0": _x0_for_core(phases, c), "cfb": cfb,
        })
    resB = run_bass_kernel_spmd(ncB, in_maps, core_ids=list(range(NCORES)))

    M = np.zeros((N, N), np.complex64)
    for c in range(NCORES):
        o = resB.results[c]["xout"]
        cols = slice(c * COLS, (c + 1) * COLS)
        for blk in range(4):
            re = o[:, (blk * 2 + 0) * COLS:(blk * 2 + 1) * COLS]
            im = o[:, (blk * 2 + 1) * COLS:(blk * 2 + 2) * COLS]
            M[blk * 128:(blk + 1) * 128, cols] = re + 1j * im
    return M

